# revision 10
# baseline (speedup 1.0000x reference)
"""Single-head attention (B=4, S=2048, D=1024) on 8 Trainium2 NeuronCores.

Sharding: core c handles batch b = c//2, query half h = c%2 (1024 queries).
K and V are each projected for the core's OWN sequence half only and the
halves are exchanged between the core pair via AllGather (rank order
[halfA | halfB] on both cores - the identity key permutation).

Math notes (exact rewrites of the reference):
  - scores row-softmax is invariant to adding a per-row constant, so the
    K-projection bias `bk` drops out entirely.
  - attn rows sum to 1, so the V bias `bv` is a constant additive term on
    the output: out = attn @ V_nobias + bv.
  - softmax is computed without max-subtraction: scores/32 has |s| < ~4 for
    this problem (checked host-side), exp() is well-conditioned there.

Precision (identical noise recipe to the 216us baseline, 1.9e-2 l2):
  Kt fully fp8e4m3; Qt fp8 on e-tiles 0-5, bf16 on 6-7. Scores per
  512-query chunk = 3 fp8 DoubleRow matmuls + 2 mixed fp8xbf16 matmuls.
  Everything else (projections, exp, attn@V) is bf16 with fp32 accum.

v2 schedule changes (from perfetto trace of the 211.6us baseline):
  - Inputs ride FOUR trigger queues (sync/gpsimd/scalar/vector) instead of
    two; tensors are host-packed [128, 8192] row-major so each 512KB load
    is one contiguous-AP DMA. Baseline: wk et0 landed ~21us (2-queue
    serialization); now the Kproj prefix lands ~11us.
  - Warm-up matmul count cut 40 -> 12 (sized to the new landing time).
  - Scores are computed TRANSPOSED: scoresT[k,q] with K stationary and Q
    moving (same operands, swapped matmul roles -> identical fp8 noise).
    The exp output expT[k,q] is then directly the stationary operand of
    attn@V - the 128 PE transposes (~7us) and 128 DVE psum->sbuf copies
    of the baseline disappear. Softmax denominators come from a width-1
    ones-column matmul riding the same LDWEIGHTS as the attn@V chunks
    (~60cyc each, all 8 qtiles' denominators accumulate in ONE psum bank).
  - Output DMAs alternate over the sync+gpsimd queues (both idle by then);
    the baseline's single-queue backlog made the final drain 4.9us.

Device pipeline per core (PSUM accumulation fp32):
  Phase A: short warm-up; zero-payload warm-up collective absorbs the
           one-time CC init; Kt own half -> fp8 AllGather; V[s,e] own half
           -> bf16 AllGather; Qt[e,q] (+bq via ACT bias on fp8 tiles, DVE
           add on bf16 tiles).
  Phase B: per 128-key tile x 512-query chunk: scoresT psum (3 DR + 2
           mixed) -> exp(s/32) on ACT -> expT in SBUF. Then per qtile:
           out = expT[:,qt].T @ V (16 ktiles x {512,512,1(ones)} chunks);
           DVE recip of den psum; ACT applies 1/rowsum; DVE adds bv;
           sync/gpsimd trigger DMA out.
"""

import numpy as np
import ml_dtypes

from contextlib import ExitStack

import concourse.bass as bass
import concourse.mybir as mybir
import concourse.tile as tile
from concourse import bacc

BF16 = mybir.dt.bfloat16
F8 = mybir.dt.float8e4
F32 = mybir.dt.float32
NPBF16 = ml_dtypes.bfloat16

B, S, D = 4, 2048, 1024
NCORES = 8
SQ = S // 2            # queries / own-half keys per core
P = 128                # partitions
NDT = D // P           # 8 d-tiles (input feature dim)
NET = D // P           # 8 e-tiles (projected dim)
NST = S // P           # 16 key tiles
NQT = SQ // P          # 8 query tiles per core
NKC = S // 512         # 4 key chunks of 512
NQC = SQ // 512        # 2 query chunks of 512
NEC = D // 512         # 2 embed chunks of 512
SCALE = 1.0 / 32.0     # 1/sqrt(D)

NF8 = 6                # e-tiles of the scores contraction in fp8 (even)
NB16 = NET - NF8       # e-tiles kept bf16
WARMUP = 12

AF = mybir.ActivationFunctionType
DR = mybir.MatmulPerfMode.DoubleRow

_PROGRAM = None


def _build_program():
    nc = bacc.Bacc(
        "TRN2", target_bir_lowering=False, debug=False, num_devices=NCORES
    )
    # all big inputs host-packed [P, 8192] so column slices are contiguous
    xq_d = nc.dram_tensor("xq", [P, NDT * SQ], BF16, kind="ExternalInput")
    wq_d = nc.dram_tensor("wq", [P, NET * D], BF16, kind="ExternalInput")
    wk_d = nc.dram_tensor("wk", [P, NET * D], BF16, kind="ExternalInput")
    wv_d = nc.dram_tensor("wv", [P, NEC * NDT * 512], BF16, kind="ExternalInput")
    bq_d = nc.dram_tensor("bq", [P, NET], F32, kind="ExternalInput")
    bv_d = nc.dram_tensor("bv", [1, D], F32, kind="ExternalInput")
    out_d = nc.dram_tensor("out", [SQ, D], F32, kind="ExternalOutput")

    with tile.TileContext(nc) as tc, ExitStack() as ctx:
        consts = ctx.enter_context(tc.tile_pool(name="consts", bufs=1))
        xpool = ctx.enter_context(tc.tile_pool(name="xpool", bufs=1))
        # bufs=3: wq must NOT alias wk's buffer - its early-issued DMA
        # trigger would otherwise block a queue until Kproj finishes
        wpool = ctx.enter_context(tc.tile_pool(name="wpool", bufs=3))
        stage = ctx.enter_context(tc.tile_pool(name="stage", bufs=1))
        proj = ctx.enter_context(tc.tile_pool(name="proj", bufs=1))
        bpool = ctx.enter_context(tc.tile_pool(name="bpool", bufs=2))
        dpool = ctx.enter_context(tc.tile_pool(name="dpool", bufs=1, space="DRAM"))
        ps = ctx.enter_context(tc.tile_pool(name="ps", bufs=5, space="PSUM"))
        pst = ctx.enter_context(tc.tile_pool(name="pst", bufs=2, space="PSUM"))
        psden = ctx.enter_context(tc.tile_pool(name="psden", bufs=1, space="PSUM"))

        # --- PE warm-up: dummy matmuls ramp the PE clock p-state while the
        # first inputs land. gpsimd's queue boots ~1.5us before vector's,
        # so the memset there unblocks the first LDWEIGHTS earlier. ---
        warm = consts.tile([P, 640], BF16)
        nc.gpsimd.memset(warm[:], 0.0)

        # tiny warm-up collective: absorbs the one-time CC init (measured
        # ~12-20us from kernel start regardless of payload; without it the
        # first REAL mesh starts later and runs longer). Input staged
        # dram->dram from an ExternalInput; vector's queue so sync/gpsimd
        # start their big input loads immediately.
        pairs = [[2 * i, 2 * i + 1] for i in range(NCORES // 2)]
        ccw_in = dpool.tile([1, NET], F32, tag="ccw_in")
        ccw_out = dpool.tile([2, 1, NET], F32, tag="ccw_out")
        nc.scalar.dma_start(out=ccw_in[:], in_=bq_d[0:1, :])
        nc.gpsimd.collective_compute(
            "AllGather", mybir.AluOpType.bypass, replica_groups=pairs,
            ins=[ccw_in[:]], outs=[ccw_out[:]],
        )
        for _ in range(WARMUP):
            wps = pst.tile([P, 512], F32, tag="warmps")
            nc.tensor.matmul(
                wps[:], lhsT=warm[:, 512:640], rhs=warm[:, 0:512],
                start=True, stop=True,
            )

        # --- input loads spread over the three DMA-capable trigger queues
        # (sync / gpsimd / scalar). xq rides 8 per-dt 256KB loads round-
        # robined so the dt-outer Kproj below can start consuming tiles
        # ~10.5us in; weights ride 512KB contiguous slices, first-needed
        # first. Queue discipline afterwards: gpsimd/sync take collective
        # triggers + gather returns + output stores; scalar drains
        # projection PSUMs.
        xq_sb = xpool.tile([P, NDT * SQ], BF16)
        wk_sb = wpool.tile([P, NET * D], BF16, tag="w")
        wv_sb = wpool.tile([P, NEC * NDT * 512], BF16, tag="w")
        wq_sb = wpool.tile([P, NET * D], BF16, tag="w")
        H = NET * D // 4  # 2048 cols = 512KB per load

        def wslice(dst, src, i):
            return dst[:, i * H:(i + 1) * H], src[:, i * H:(i + 1) * H]

        def xslice(dt):
            return (xq_sb[:, dt * SQ:(dt + 1) * SQ],
                    xq_d[:, dt * SQ:(dt + 1) * SQ])

        loads = {
            nc.sync:   [wslice(wk_sb, wk_d, 0), xslice(2), xslice(5),
                        wslice(wk_sb, wk_d, 2), wslice(wq_sb, wq_d, 1),
                        wslice(wq_sb, wq_d, 3)],
            nc.gpsimd: [xslice(0), xslice(3), xslice(6),
                        wslice(wk_sb, wk_d, 1), wslice(wv_sb, wv_d, 0),
                        wslice(wv_sb, wv_d, 1), wslice(wq_sb, wq_d, 2)],
            nc.scalar: [xslice(1), xslice(4), xslice(7),
                        wslice(wk_sb, wk_d, 3), wslice(wq_sb, wq_d, 0),
                        wslice(wv_sb, wv_d, 2), wslice(wv_sb, wv_d, 3)],
        }
        for q, ls in loads.items():
            for d_, s_ in ls:
                q.dma_start(out=d_, in_=s_)

        # --- constants (emitted after the startup-critical DMAs) ---
        bq_sb = consts.tile([P, NET], F32)
        nc.scalar.dma_start(out=bq_sb[:], in_=bq_d[:])
        bv_sb = consts.tile([P, D], F32)
        nc.sync.dma_start(out=bv_sb[:], in_=bv_d[:].to_broadcast([P, D]))
        ones_sb = consts.tile([P, 1], BF16)
        nc.gpsimd.memset(ones_sb[:], 1.0)

        # projected tensors (full-sequence K/V assembled from both halves).
        # K is fully fp8 (one small mesh); the precision anchor lives on the
        # Q side: Qt keeps et 6,7 in bf16 (mixed fp8 x bf16 matmuls run at
        # full PE rate).
        kt8_sb = proj.tile([P, NET, S], F8)     # Kt[e,s], all e-tiles
        v_sb = proj.tile([P, NST, D], BF16)     # V[s,e], s-tile major
        qt8_sb = proj.tile([P, NF8, SQ], F8)    # Qt[e,q], et 0..NF8-1
        qtb_sb = proj.tile([P, NB16, SQ], BF16)  # Qt[e,q], et NF8..7
        expT_sb = proj.tile([P, NST, SQ], BF16)  # exp(scoresT)[k,q]

        # DRAM staging for the pair exchanges
        kv_k8 = dpool.tile([P, NET, SQ], F8, tag="kv_k8")
        kv_k8o = dpool.tile([2, P, NET, SQ], F8, tag="kv_k8o")
        kv_v = dpool.tile([P, NST // 2, D], BF16, tag="kv_v")
        kv_vo = dpool.tile([2, P, NST // 2, D], BF16, tag="kv_vo")

        # K/V gather-return + output DMAs round-robin over sync+gpsimd
        trig = [nc.sync, nc.gpsimd]
        _t = [0]

        def dma(out, in_):
            trig[_t[0] % len(trig)].dma_start(out=out, in_=in_)
            _t[0] += 1

        # --- phase A1: Kt own half -> one fp8 exchange. dt-OUTER loop so
        # the first et-group's matmuls start as soon as wk et0 + xq dt0
        # land (~10.5us) and consume the remaining xq tiles as they arrive,
        # instead of waiting for the slowest xq load.
        for et in range(NET):
            psums = [ps.tile([P, 512], F32, tag="psum", name=f"kp{et}_{i}") for i in range(NQC)]
            for dt in range(NDT):
                for qc in range(NQC):
                    nc.tensor.matmul(
                        psums[qc][:],
                        lhsT=wk_sb[:, et * D + dt * P: et * D + (dt + 1) * P],
                        rhs=xq_sb[:, dt * SQ + qc * 512: dt * SQ + qc * 512 + 512],
                        start=(dt == 0),
                        stop=(dt == NDT - 1),
                    )
            for qc in range(NQC):
                kh = stage.tile([P, 512], F8, tag="kh8", bufs=4)
                nc.scalar.copy(kh[:], psums[qc][:])
                nc.scalar.dma_start(
                    out=kv_k8[:, et, qc * 512:(qc + 1) * 512], in_=kh[:]
                )
        nc.gpsimd.collective_compute(
            "AllGather", mybir.AluOpType.bypass, replica_groups=pairs,
            ins=[kv_k8[:]], outs=[kv_k8o[:]],
        )
        # gathered-K return DMAs, split per e-tile across both trigger
        # queues (trigger issue is ~0.7us apiece). Emitted before the V
        # trigger so the returns get the fabric before V's ring DMAs.
        for r in range(2):
            for et in range(NET):
                dma(kt8_sb[:, et, SQ * r:SQ * (r + 1)], kv_k8o[r][:, et, :])

        # --- phase A2: V own half -> exchange (consumed last) ---
        for st in range(NST // 2):
            v_hst = stage.tile([P, D], BF16, tag="vh", bufs=4, name=f"vh{st}")
            for ec in range(NEC):
                psum = ps.tile([P, 512], F32)
                for dt in range(NDT):
                    nc.tensor.matmul(
                        psum[:],
                        lhsT=xq_sb[:, dt * SQ + st * P: dt * SQ + (st + 1) * P],
                        rhs=wv_sb[
                            :, ec * NDT * 512 + dt * 512: ec * NDT * 512 + dt * 512 + 512
                        ],
                        start=(dt == 0),
                        stop=(dt == NDT - 1),
                    )
                nc.scalar.copy(v_hst[:, ec * 512:(ec + 1) * 512], psum[:])
            nc.scalar.dma_start(out=kv_v[:, st, :], in_=v_hst[:])
        nc.gpsimd.collective_compute(
            "AllGather", mybir.AluOpType.bypass, replica_groups=pairs,
            ins=[kv_v[:]], outs=[kv_vo[:]],
        )
        # rank r's half occupies s-tiles [r*8, r*8+8); returns split per
        # s-tile across both trigger queues
        for r in range(2):
            for st in range(NST // 2):
                dma(v_sb[:, (NST // 2) * r + st, :], kv_vo[r][:, st, :])

        # --- phase A3: Qt for this core's queries (bias fused via ACT) ---
        for et in range(NET):
            for qc in range(NQC):
                psum = ps.tile([P, 512], F32)
                for dt in range(NDT):
                    nc.tensor.matmul(
                        psum[:],
                        lhsT=wq_sb[:, et * D + dt * P: et * D + (dt + 1) * P],
                        rhs=xq_sb[:, dt * SQ + qc * 512: dt * SQ + qc * 512 + 512],
                        start=(dt == 0),
                        stop=(dt == NDT - 1),
                    )
                if et < NF8:
                    nc.scalar.activation(
                        qt8_sb[:, et, qc * 512:(qc + 1) * 512], psum[:],
                        AF.Identity, bias=bq_sb[:, et:et + 1], scale=1.0,
                    )
                else:
                    # bf16 drains ride the idle DVE so the first scores
                    # group doesn't wait on the serial ACT queue
                    nc.vector.tensor_scalar_add(
                        qtb_sb[:, et - NF8, qc * 512:(qc + 1) * 512],
                        psum[:], bq_sb[:, et:et + 1],
                    )

        # --- phase B1: scoresT = K @ Qt (transposed orientation: k on
        # partitions, q on free axis). exp output lands directly in the
        # layout attn@V needs as its stationary operand. All 32 (kt,qc)
        # chunks run before the first attn@V so the V mesh + returns always
        # land in time despite ~20us of run-to-run CC jitter.
        for kt in range(NST):
            for qc in range(NQC):
                psum = ps.tile([P, 512], F32)
                for a in range(NF8 // 2):
                    nc.tensor.matmul(
                        psum[:],
                        lhsT=kt8_sb[:, 2 * a:2 * a + 2, kt * P:(kt + 1) * P],
                        rhs=qt8_sb[:, 2 * a:2 * a + 2, qc * 512:(qc + 1) * 512],
                        start=(a == 0),
                        stop=False,
                        perf_mode=DR,
                    )
                for e in range(NB16):
                    # mixed dtype: fp8 stationary Kt x bf16 moving Qt
                    nc.tensor.matmul(
                        psum[:],
                        lhsT=kt8_sb[:, NF8 + e, kt * P:(kt + 1) * P],
                        rhs=qtb_sb[:, e, qc * 512:(qc + 1) * 512],
                        start=False,
                        stop=(e == NB16 - 1),
                    )
                nc.scalar.activation(
                    expT_sb[:, kt, qc * 512:(qc + 1) * 512], psum[:],
                    AF.Exp, bias=0.0, scale=SCALE,
                )

        # --- phase B2: out = expT[:,qt].T @ [V | ones]. The ones column
        # rides the same stationary operand as the 512-wide V chunks, so
        # all 8 qtiles' softmax denominators accumulate in ONE psum bank
        # (column qt) for ~60 cycles apiece.
        den_ps = psden.tile([P, NQT], F32)

        def finish_chunk(qt, sl, psum, recip, out_sb):
            nc.scalar.activation(
                out_sb[:, sl], psum[:], AF.Identity, bias=0.0, scale=recip[:],
            )
            nc.vector.tensor_add(out_sb[:, sl], out_sb[:, sl], bv_sb[:, sl])
            dma(out_d[qt * P:(qt + 1) * P, sl], out_sb[:, sl])

        def emit_out(qt):
            recip = bpool.tile([P, 1], F32, tag="recip")
            out_sb = bpool.tile([P, D], F32, tag="osb")
            # the V chunks and the ones column share each ks's stationary
            # operand, so the den matmul costs only its ~60-cycle floor
            e0 = ps.tile([P, 512], F32, tag="psum", name=f"av0_{qt}")
            e1 = ps.tile([P, 512], F32, tag="psum", name=f"av1_{qt}")
            for ks in range(NST):
                lhsT = expT_sb[:, ks, qt * P:(qt + 1) * P]
                nc.tensor.matmul(
                    e0[:], lhsT=lhsT, rhs=v_sb[:, ks, 0:512],
                    start=(ks == 0), stop=(ks == NST - 1),
                )
                nc.tensor.matmul(
                    e1[:], lhsT=lhsT, rhs=v_sb[:, ks, 512:1024],
                    start=(ks == 0), stop=(ks == NST - 1),
                )
                nc.tensor.matmul(
                    den_ps[:, qt:qt + 1], lhsT=lhsT, rhs=ones_sb[:, 0:1],
                    start=(ks == 0), stop=(ks == NST - 1),
                )
            nc.vector.reciprocal(recip[:], den_ps[:, qt:qt + 1])
            finish_chunk(qt, slice(0, 512), e0, recip, out_sb)
            finish_chunk(qt, slice(512, 1024), e1, recip, out_sb)

        def emit_out_last(qt):
            # the final qtile drains in 256-wide chunk-sequential chains:
            # the serial ACT->DVE->DMA tail after the very last matmul
            # halves, and each chunk's tail overlaps the next chunk's MMs
            recip = bpool.tile([P, 1], F32, tag="recip")
            out_sb = bpool.tile([P, D], F32, tag="osb")
            for ec in range(4):
                psum = pst.tile([P, 256], F32, tag="warmps")
                for ks in range(NST):
                    lhsT = expT_sb[:, ks, qt * P:(qt + 1) * P]
                    nc.tensor.matmul(
                        psum[:], lhsT=lhsT,
                        rhs=v_sb[:, ks, ec * 256:(ec + 1) * 256],
                        start=(ks == 0), stop=(ks == NST - 1),
                    )
                    if ec == 0:
                        nc.tensor.matmul(
                            den_ps[:, qt:qt + 1], lhsT=lhsT,
                            rhs=ones_sb[:, 0:1],
                            start=(ks == 0), stop=(ks == NST - 1),
                        )
                if ec == 0:
                    nc.vector.reciprocal(recip[:], den_ps[:, qt:qt + 1])
                finish_chunk(qt, slice(ec * 256, (ec + 1) * 256), psum,
                             recip, out_sb)

        for qt in range(NQT - 1):
            emit_out(qt)
        emit_out_last(NQT - 1)

    nc.compile()
    return nc


def get_program():
    global _PROGRAM
    if _PROGRAM is None:
        _PROGRAM = _build_program()
    return _PROGRAM


def make_in_maps(x, Wq, bq, Wk, bk, Wv, bv):
    """Host-side sharding/layout prep. bk is intentionally unused (softmax
    shift invariance along the key axis)."""
    x = np.asarray(x, dtype=np.float32)

    def et_major(w):
        # W.T is [d, e]; pack as [p, et*1024 + dt*128 + j] so each et-slice
        # is contiguous per partition row:
        # out[p, et*D + dt*128 + j] = W.T[dt*128+p, et*128+j]
        wt = np.asarray(w, dtype=np.float32).T.astype(NPBF16)
        return np.ascontiguousarray(
            wt.reshape(NDT, P, NET, P).transpose(1, 2, 0, 3).reshape(P, NET * D)
        )

    wq_t = et_major(Wq)
    wk_t = et_major(Wk)
    # wv packed ec-major: out[p, ec*4096 + dt*512 + j] = Wv.T[dt*128+p, ec*512+j]
    wvT = np.asarray(Wv, dtype=np.float32).T.astype(NPBF16)
    wv_t = np.ascontiguousarray(
        wvT.reshape(NDT, P, NEC, 512).transpose(1, 2, 0, 3).reshape(P, NEC * NDT * 512)
    )
    bq2 = np.ascontiguousarray(
        np.asarray(bq, dtype=np.float32).reshape(NET, P).T
    )
    bv2 = np.asarray(bv, dtype=np.float32).reshape(1, D)

    in_maps = []
    # xq packed [p, dt*SQ + q] = x.T[dt*128+p, q] per half
    xts = [np.ascontiguousarray(x[b].T.astype(NPBF16)) for b in range(B)]
    for c in range(NCORES):
        b, h = divmod(c, 2)
        xh = xts[b][:, h * SQ:(h + 1) * SQ]  # [D, SQ]
        xq = np.ascontiguousarray(
            xh.reshape(NDT, P, SQ).transpose(1, 0, 2).reshape(P, NDT * SQ)
        )
        in_maps.append({
            "xq": xq,
            "wq": wq_t, "wk": wk_t, "wv": wv_t,
            "bq": bq2, "bv": bv2,
        })
    return in_maps


def assemble(results):
    out = np.empty((B, S, D), dtype=np.float32)
    for c in range(NCORES):
        b, h = divmod(c, 2)
        out[b, h * SQ:(h + 1) * SQ, :] = results[c]["out"]
    return out


def kernel(x, Wq, bq, Wk, bk, Wv, bv, _trace=False, _trace_kwargs=None):
    from concourse.bass_utils import run_bass_kernel_spmd

    nc = get_program()
    in_maps = make_in_maps(x, Wq, bq, Wk, bk, Wv, bv)
    res = run_bass_kernel_spmd(
        nc, in_maps, list(range(NCORES)), trace=_trace, **(_trace_kwargs or {})
    )
    out = assemble(res.results)
    if _trace:
        kernel.last_results = res
    return out
